# revision 31
# baseline (speedup 1.0000x reference)
"""Trainium2 Bass kernel for nn_DMLoss (contour matching loss), 8-core data parallel.

v4: block-diagonal bf16 split-precision matmuls with the C1 rounding offset
folded in as extra contraction rows, XBAR DMA-transposed one-hots, combined
packed argmin reduce, engine-balanced chain, piecewise prep DMA overlap.

Per instance (P=128 points, TIME=10):
  item1: nearest of 1280 interpolated gt points per pred point.  Segment n
    spans A_n = gt[n-1]..gt[n]; g' = 10*u - 0.5 with u = <p-A,D>/|D|^2; best
    discrete t = clamp(round(g'), 0, 9); dist^2 = |p-A|^2 + (e/100)*t*(t-2g').
    TensorE produces grids g', opsC1 = SC*(2<p,A> - |A|^2) + C1 and
    erep = -SC*e/100 (bf16 hi/lo split rows, C1 = C1H + C1L exactly).
    dq = (q*erep + opsC1) - CQ lands exactly on the 128-grid; pk = dq - n
    packs quantized distance + segment index; reduce-max = argmin;
    exact 0/1 one-hots gather segment data via XBAR transpose + bf16 matmul.
  item2: same machinery without interpolation (nearest pred per key point).

Output per core: [sum_loss1, sum_loss2]; host divides by counts and combines.
"""
import sys

for _p in ("/opt/trn_rl_repo",):
    if _p not in sys.path:
        sys.path.insert(0, _p)

import numpy as np

import concourse.bass as bass
import concourse.tile as tile
from concourse import bacc, mybir
from concourse.bass_utils import run_bass_kernel_spmd

dt = mybir.dt
Alu = mybir.AluOpType
Ax = mybir.AxisListType
Act = mybir.ActivationFunctionType
f32 = np.float32

N_CORES = 8
N, P = 256, 128
G = N // N_CORES          # instances per core = 32
BG = 4                    # instances per block
NB = G // BG              # 8 blocks
SC = 131072.0             # distance scale (quantum = 128/SC = 2^-10)
SHIFT = 48.0
BETA = 0.25               # smooth-l1 beta = 1/STRIDE
CQ = float(2 ** 30 + 2 ** 25)       # 1107296256
C1 = CQ - SC * SHIFT                # 1101004800
C1H = 1098907648.0                  # bf16-exact hi part of C1
C1L = 2097152.0                     # bf16-exact lo part (C1H + C1L == C1)
M23 = 8388608.0

# SPL slab regions, 10 row-types per operand (k = 10*i + t within a block):
#   lhsT: [x_hi, y_hi, x_lo, y_lo, x_hi2, y_hi2, 1, 1, 1, 1]
#   rhs:  [u_hi, v_hi, u_hi2, v_hi2, u_lo, v_lo, c_hi, c_lo, w8, w9]
# pairing t: xh*uh + yh*vh + xl*uh + yl*vh + xh*ul + yh*vl + ch + cl + w8 + w9
P0 = 0
K0 = 10
RG = 20         # w8 = w9 = 0
RO = 30         # w8, w9 = C1H, C1L
RQ = 40         # w8, w9 = C1H, C1L
RE = 50         # [z z z z z z er_hi er_lo z z]
TB = 60         # ax_hi ay_hi dx_hi dy_hi ox_hi oy_hi
NSLAB = 74
KB = 10 * BG    # block contraction rows = 40


def _build(nc, pc_d, po_d, gc_d, gk_d, mk_d, out_d):
    FP = dt.float32
    BF = dt.bfloat16

    with tile.TileContext(nc) as tc:
        with (
            tc.tile_pool(name="const", bufs=1) as cpool,
            tc.tile_pool(name="prep", bufs=1) as prep,
            tc.tile_pool(name="oper", bufs=1) as oper,
            tc.tile_pool(name="main", bufs=3) as main,
            tc.tile_pool(name="keep", bufs=1) as keep,
        ):
            V, Gp, S = nc.vector, nc.gpsimd, nc.scalar

            # ---------------- constants ----------------
            iota_i = cpool.tile([128, 128], dt.int32)
            Gp.iota(iota_i[:], pattern=[[1, 128]], channel_multiplier=0)
            iotaF = cpool.tile([128, 128], FP, tag="iotaF")
            V.tensor_copy(iotaF[:], iota_i[:])
            iotaC_i = cpool.tile([128, 1], dt.int32)
            Gp.iota(iotaC_i[:], pattern=[[0, 1]], channel_multiplier=1)
            iotaC = cpool.tile([128, 1], FP, tag="iotaC")
            V.tensor_copy(iotaC[:], iotaC_i[:])
            ident32F = cpool.tile([32, 32], FP, tag="ident32F")
            V.tensor_scalar(ident32F[:], iotaF[0:32, 0:32], iotaC[0:32], None, Alu.is_equal)
            onesc = cpool.tile([128, 1], FP, tag="onesc")
            Gp.memset(onesc[:], 1.0)

            # zeros region for rhs backfill
            zero_d = nc.dram_tensor("zeros", [KB, 32, 512], BF)
            ztile = prep.tile([KB, 512], BF, tag="ztile")
            V.memset(ztile[:], 0.0)
            nc.sync.dma_start(zero_d[:, 0, :], ztile[:])
            nc.sync.dma_start(zero_d[:, 1, :], zero_d[:, 0, :])
            nc.sync.dma_start(zero_d[:, 2:4, :], zero_d[:, 0:2, :])
            nc.sync.dma_start(zero_d[:, 4:8, :], zero_d[:, 0:4, :])
            nc.sync.dma_start(zero_d[:, 8:16, :], zero_d[:, 0:8, :])
            nc.sync.dma_start(zero_d[:, 16:32, :], zero_d[:, 0:16, :])

            # ---------------- contiguous input loads ----------------
            pc_i = prep.tile([32, 128, 2], FP, tag="pc_i")
            po_i = prep.tile([32, 128, 2], FP, tag="po_i")
            gc_i = prep.tile([32, 128, 2], FP, tag="gc_i")
            gk_i = prep.tile([32, 128, 2], FP, tag="gk_i")
            mk_i = prep.tile([32, 128], FP, tag="mk_i")
            a_i = prep.tile([32, 128, 2], FP, tag="a_i")
            nc.sync.dma_start(pc_i[:], pc_d[:, :, :])
            nc.sync.dma_start(po_i[:], po_d[:, :, :])
            nc.sync.dma_start(gc_i[:], gc_d[:, :, :])
            nc.scalar.dma_start(gk_i[:], gk_d[:, :, :])
            nc.scalar.dma_start(mk_i[:], mk_d[:, :])
            nc.sync.dma_start(a_i[:, 1:128, :], gc_d[:, 0:127, :])
            nc.scalar.dma_start(a_i[:, 0:1, :], gc_d[:, 127:128, :])

            # ---------------- SPL slabs + fine-grained stores ----------------
            SPL = prep.tile([32, NSLAB, 128], BF, tag="SPL")
            slab_d = nc.dram_tensor("slabs", [32, NSLAB, 128], BF)

            def split_pair(src_, s_hi, s_lo):
                # contiguous bf16 writes; channel-major strided reads
                srcT = src_.rearrange("g q c -> g c q")
                S.activation(SPL[:, s_hi:s_hi + 2, :], srcT, Act.Copy)
                if s_lo is not None:
                    V.tensor_tensor(SPL[:, s_lo:s_lo + 2, :], srcT,
                                    SPL[:, s_hi:s_hi + 2, :], Alu.subtract)

            def split_one(src_, s_hi, s_lo):
                S.activation(SPL[:, s_hi, :], src_, Act.Copy)
                if s_lo is not None:
                    V.tensor_tensor(SPL[:, s_lo, :], src_, SPL[:, s_hi, :], Alu.subtract)

            def split_pair_scaled(src_, sc, s_hi, s_lo):
                srcT = src_.rearrange("g q c -> g c q")
                S.activation(SPL[:, s_hi:s_hi + 2, :], srcT, Act.Copy, scale=sc)
                V.scalar_tensor_tensor(SPL[:, s_lo:s_lo + 2, :], srcT, sc,
                                       SPL[:, s_hi:s_hi + 2, :], Alu.mult, Alu.subtract)

            def split_one_scaled(src_, sc, s_hi, s_lo):
                S.activation(SPL[:, s_hi, :], src_, Act.Copy, scale=sc)
                V.scalar_tensor_tensor(SPL[:, s_lo, :], src_, sc,
                                       SPL[:, s_hi, :], Alu.mult, Alu.subtract)

            # ---- phase A: pc/gk-dependent slabs (no gc geometry needed) ----
            Gp.memset(SPL[:, P0 + 6:P0 + 10, :], 1.0)
            Gp.memset(SPL[:, K0 + 6:K0 + 10, :], 1.0)
            Gp.memset(SPL[:, RQ + 8, :], C1H)
            Gp.memset(SPL[:, RQ + 9, :], C1L)
            split_pair(pc_i[:], P0 + 0, P0 + 2)
            split_pair(gk_i[:], K0 + 0, K0 + 2)
            V.tensor_copy(SPL[:, P0 + 4:P0 + 6, :], SPL[:, P0 + 0:P0 + 2, :])
            V.tensor_copy(SPL[:, K0 + 4:K0 + 6, :], SPL[:, K0 + 0:K0 + 2, :])
            psq = prep.tile([32, 128, 2], FP, tag="psq")
            Gp.tensor_tensor(psq[:], pc_i[:], pc_i[:], Alu.mult)
            zP = prep.tile([32, 128], FP, tag="zP")
            Gp.tensor_tensor(zP[:], psq[:, :, 0], psq[:, :, 1], Alu.add)
            split_pair_scaled(pc_i[:], 2.0 * SC, RQ + 0, RQ + 4)
            split_one_scaled(zP[:], -SC, RQ + 6, RQ + 7)
            V.tensor_copy(SPL[:, RQ + 2:RQ + 4, :], SPL[:, RQ + 0:RQ + 2, :])
            nc.sync.dma_start(slab_d[:, 0:20, :], SPL[:, 0:20, :])
            nc.scalar.dma_start(slab_d[:, RQ:RQ + 10, :], SPL[:, RQ:RQ + 10, :])

            # ---- phase B: gc geometry ----
            d_i = prep.tile([32, 128, 2], FP, tag="d_i")
            V.tensor_tensor(d_i[:], gc_i[:], a_i[:], Alu.subtract)
            dsq = prep.tile([32, 128, 2], FP, tag="dsq")
            Gp.tensor_tensor(dsq[:], d_i[:], d_i[:], Alu.mult)
            e = prep.tile([32, 128], FP, tag="e")
            Gp.tensor_tensor(e[:], dsq[:, :, 0], dsq[:, :, 1], Alu.add)
            einv = prep.tile([32, 128], FP, tag="einv")
            V.reciprocal(einv[:], e[:])
            t_ad = prep.tile([32, 128, 2], FP, tag="t_ad")
            Gp.tensor_tensor(t_ad[:], a_i[:], d_i[:], Alu.mult)
            a2 = prep.tile([32, 128], FP, tag="a2")
            Gp.tensor_tensor(a2[:], t_ad[:, :, 0], t_ad[:, :, 1], Alu.add)
            asq = prep.tile([32, 128, 2], FP, tag="asq")
            Gp.tensor_tensor(asq[:], a_i[:], a_i[:], Alu.mult)
            zA = prep.tile([32, 128], FP, tag="zA")
            Gp.tensor_tensor(zA[:], asq[:, :, 0], asq[:, :, 1], Alu.add)

            Gp.memset(SPL[:, RE + 0:RE + 6, :], 0.0)
            Gp.memset(SPL[:, RE + 8:RE + 10, :], 0.0)
            split_one_scaled(e[:], -SC / 100.0, RE + 6, RE + 7)
            nc.sync.dma_start(slab_d[:, RE:RE + 10, :], SPL[:, RE:RE + 10, :])

            r_01 = prep.tile([32, 128, 2], FP, tag="r_01")
            V.scalar_tensor_tensor(r_01[:, :, 0], d_i[:, :, 0], 10.0, einv[:], Alu.mult, Alu.mult)
            V.scalar_tensor_tensor(r_01[:, :, 1], d_i[:, :, 1], 10.0, einv[:], Alu.mult, Alu.mult)
            r2 = prep.tile([32, 128], FP, tag="r2")
            V.scalar_tensor_tensor(r2[:], a2[:], -10.0, einv[:], Alu.mult, Alu.mult)
            Gp.memset(SPL[:, RG + 8:RG + 10, :], 0.0)
            split_pair(r_01[:], RG + 0, RG + 4)
            split_one(r2[:], RG + 6, RG + 7)
            V.tensor_copy(SPL[:, RG + 2:RG + 4, :], SPL[:, RG + 0:RG + 2, :])
            nc.scalar.dma_start(slab_d[:, RG:RG + 10, :], SPL[:, RG:RG + 10, :])

            Gp.memset(SPL[:, RO + 8, :], C1H)
            Gp.memset(SPL[:, RO + 9, :], C1L)
            split_pair_scaled(a_i[:], 2.0 * SC, RO + 0, RO + 4)
            split_one_scaled(zA[:], -SC, RO + 6, RO + 7)
            V.tensor_copy(SPL[:, RO + 2:RO + 4, :], SPL[:, RO + 0:RO + 2, :])
            nc.sync.dma_start(slab_d[:, RO:RO + 10, :], SPL[:, RO:RO + 10, :])

            # ---- table region ----
            split_pair(a_i[:], TB + 0, None)
            split_pair(d_i[:], TB + 2, None)
            split_pair(po_i[:], TB + 12, None)
            V.tensor_copy(SPL[:, TB + 4:TB + 6, :], SPL[:, RG + 0:RG + 5:4, :])
            V.tensor_copy(SPL[:, TB + 6:TB + 8, :], SPL[:, RG + 1:RG + 6:4, :])
            V.tensor_copy(SPL[:, TB + 8:TB + 10, :], SPL[:, RG + 6:RG + 8, :])
            V.tensor_copy(SPL[:, TB + 10:TB + 12, :], SPL[:, P0 + 0:P0 + 2, :])


            # lhsA[t, s, b, p] = slab (s=0 pred, s=1 key); rhsA[t, r, b, n]
            lhsA = oper.tile([KB, 2, 8, 128], BF, tag="lhsA")
            rhsA = oper.tile([KB, 4, 8, 512], BF, tag="rhsA")
            nc.sync.dma_start(
                rhsA[:], zero_d[:, :, :].rearrange("t (r b) n -> t r b n", r=4))

            for i in range(BG):
                for s in range(2):
                    eng = nc.sync if ((i + s) % 2 == 0) else nc.scalar
                    eng.dma_start(
                        lhsA[10 * i:10 * i + 10, s, :, :],
                        slab_d[i:32:4, 10 * s:10 * s + 10, :].rearrange("b t p -> t b p"),
                    )
                for r in range(4):
                    eng = nc.sync if ((i + r) % 2 == 0) else nc.scalar
                    eng.dma_start(
                        rhsA[10 * i:10 * i + 10, r, :, 128 * i:128 * (i + 1)],
                        slab_d[i:32:4, 20 + 10 * r:30 + 10 * r, :].rearrange("b t p -> t b p"),
                    )

            # ---------------- gather tables (one batched XBAR transpose) -----
            # in (32, 14*128) -> out stgB[n, j, g] = SPL[g, TB+j, n]
            T12 = keep.tile([128, G, 10], BF, tag="T12")
            T3c = keep.tile([128, G, 4], BF, tag="T3c")
            stgB = keep.tile([128, 14, 32], BF, tag="stgB")
            nc.scalar.dma_start_transpose(
                stgB[:], SPL[:, TB:TB + 14, :].rearrange("g j n -> g (j n)"))
            # T12 slots: [ax, ay, dx, dy, r0h, r0l, r1h, r1l, r2h, r2l] = j 0..9
            V.tensor_copy(T12[:], stgB[:, 0:10, :].rearrange("n j g -> n g j"))
            # T3c slots: [px, py, ox, oy] = j 10..13
            V.tensor_copy(T3c[:], stgB[:, 10:14, :].rearrange("n j g -> n g j"))

            # ---------------- f32 transposes for the tail --------------------
            pxP = keep.tile([128, G], FP, tag="pxP")
            pyP = keep.tile([128, G], FP, tag="pyP")
            oxP = keep.tile([128, G], FP, tag="oxP")
            oyP = keep.tile([128, G], FP, tag="oyP")
            kxP = keep.tile([128, G], FP, tag="kxP")
            kyP = keep.tile([128, G], FP, tag="kyP")
            mkP = keep.tile([128, G], FP, tag="mkP")
            with tc.tile_pool(name="ps_prep", bufs=3, space="PSUM") as ps_prep:
                for dst, src in ((pxP, pc_i[:, :, 0]), (pyP, pc_i[:, :, 1]),
                                 (oxP, po_i[:, :, 0]), (oyP, po_i[:, :, 1]),
                                 (kxP, gk_i[:, :, 0]), (kyP, gk_i[:, :, 1]),
                                 (mkP, mk_i[:])):
                    fps = ps_prep.tile([128, 32], FP, tag="tpsF")
                    nc.tensor.transpose(fps[:], src, ident32F[:])
                    S.activation(dst[:], fps[:], Act.Copy)

            exA = keep.tile([128, 16, 14], FP, tag="exA")
            exB = keep.tile([128, 16, 14], FP, tag="exB")
            tl = prep

            def tail_chunk(ex, gl, sfx):
                gs = slice(gl, gl + 16)

                def TT(name, a, bb, op, eng=V):
                    r = tl.tile([128, 16], FP, tag=sfx + name)
                    eng.tensor_tensor(r[:], a, bb, op)
                    return r

                r0 = TT("r0", ex[:, :, 4], ex[:, :, 5], Alu.add)
                r1 = TT("r1", ex[:, :, 6], ex[:, :, 7], Alu.add, Gp)
                r2t = TT("r2t", ex[:, :, 8], ex[:, :, 9], Alu.add)
                v1 = TT("v1", pxP[:, gs], r0[:], Alu.mult, Gp)
                v2 = TT("v2", pyP[:, gs], r1[:], Alu.mult)
                gst = TT("gst", v1[:], v2[:], Alu.add, Gp)
                gst = TT("gst2", gst[:], r2t[:], Alu.add)
                c2t = tl.tile([128, 16], FP, tag=sfx + "c2t")
                S.activation(c2t[:], gst[:], Act.Copy, bias=M23)
                c3t = tl.tile([128, 16], FP, tag=sfx + "c3t")
                S.activation(c3t[:], c2t[:], Act.Copy, bias=-M23)
                tst = tl.tile([128, 16], FP, tag=sfx + "tst")
                V.tensor_scalar(tst[:], c3t[:], 0.0, 9.0, Alu.max, Alu.min)
                m1 = TT("m1", tst[:], ex[:, :, 2], Alu.mult, Gp)
                tgx = tl.tile([128, 16], FP, tag=sfx + "tgx")
                V.scalar_tensor_tensor(tgx[:], m1[:], 0.1, ex[:, :, 0], Alu.mult, Alu.add)
                m2 = TT("m2", tst[:], ex[:, :, 3], Alu.mult, Gp)
                tgy = tl.tile([128, 16], FP, tag=sfx + "tgy")
                V.scalar_tensor_tensor(tgy[:], m2[:], 0.1, ex[:, :, 1], Alu.mult, Alu.add)

                def smooth_l1_sum(pred_x, pred_y, tx, ty, px_, py_, name):
                    acc = None
                    for ci, (pr, tt_, pp) in enumerate(((pred_x, tx, px_), (pred_y, ty, py_))):
                        s2fx = sfx + name + str(ci)
                        e1 = TT(name + str(ci) + "e1", tt_, pp, Alu.subtract, Gp)
                        dfe = tl.tile([128, 16], FP, tag=s2fx + "dfe")
                        V.scalar_tensor_tensor(dfe[:], e1[:], -0.25, pr, Alu.mult, Alu.add)
                        ad = tl.tile([128, 16], FP, tag=s2fx + "ad")
                        S.activation(ad[:], dfe[:], Act.Abs)
                        m = tl.tile([128, 16], FP, tag=s2fx + "m")
                        V.tensor_scalar(m[:], ad[:], BETA, None, Alu.min)
                        uu = tl.tile([128, 16], FP, tag=s2fx + "u")
                        V.scalar_tensor_tensor(uu[:], m[:], -0.5, ad[:], Alu.mult, Alu.add)
                        sl = tl.tile([128, 16], FP, tag=s2fx + "sl")
                        V.scalar_tensor_tensor(sl[:], m[:], 4.0, uu[:], Alu.mult, Alu.mult)
                        if acc is None:
                            acc = sl
                        else:
                            acc = TT(name + "acc", acc[:], sl[:], Alu.add, Gp)
                    return acc

                s1 = smooth_l1_sum(oxP[:, gs], oyP[:, gs], tgx[:], tgy[:],
                                   pxP[:, gs], pyP[:, gs], "i1")
                s2 = smooth_l1_sum(ex[:, :, 12], ex[:, :, 13], kxP[:, gs], kyP[:, gs],
                                   ex[:, :, 10], ex[:, :, 11], "i2")
                s2 = TT("s2m", s2[:], mkP[:, gs], Alu.mult)
                s1r = tl.tile([128, 1], FP, tag=sfx + "s1r")
                V.tensor_reduce(s1r[:], s1[:], Ax.X, Alu.add)
                s2r = tl.tile([128, 1], FP, tag=sfx + "s2r")
                V.tensor_reduce(s2r[:], s2[:], Ax.X, Alu.add)
                return s1r, s2r


            iotaB = iotaF[:].rearrange("p (o q) -> p o q", o=1).broadcast_to([128, BG, 128])

            ps_grid_cm = tc.tile_pool(name="ps_grid", bufs=1, space="PSUM")
            ps_d2_cm = tc.tile_pool(name="ps_d2", bufs=2, space="PSUM")
            ps_ex_cm = tc.tile_pool(name="ps_ex", bufs=2, space="PSUM")
            ps_out_cm = tc.tile_pool(name="ps_out", bufs=1, space="PSUM")
            ps_grid = ps_grid_cm.__enter__()
            ps_d2 = ps_d2_cm.__enter__()
            ps_ex = ps_ex_cm.__enter__()
            ps_out = ps_out_cm.__enter__()

            # ---------------- main loop ----------------
            for b in range(NB):
                g0 = b * BG
                gps = ps_grid.tile([128, BG, 128], FP, tag="gps")
                ops = ps_grid.tile([128, BG, 128], FP, tag="ops")
                erep = ps_grid.tile([128, BG, 128], FP, tag="erep")
                d2ps = ps_d2.tile([128, BG, 128], FP, tag="d2ps")
                gv = gps[:].rearrange("p i n -> p (i n)")
                ov = ops[:].rearrange("p i n -> p (i n)")
                ev = erep[:].rearrange("p i n -> p (i n)")
                dv = d2ps[:].rearrange("p i n -> p (i n)")
                nc.tensor.matmul(dv, lhsA[:, 1, b, :], rhsA[:, 2, b, :], start=True, stop=True)
                nc.tensor.matmul(gv, lhsA[:, 0, b, :], rhsA[:, 0, b, :], start=True, stop=True)
                nc.tensor.matmul(ov, lhsA[:, 0, b, :], rhsA[:, 1, b, :], start=True, stop=True)
                nc.tensor.matmul(ev, lhsA[:, 0, b, :], rhsA[:, 3, b, :], start=True, stop=True)

                # -------- item2 chain first (short, fills S/G early) --------
                dq2 = main.tile([128, BG, 128], FP, tag="dq2")
                S.activation(dq2[:], d2ps[:], Act.Copy, bias=-CQ)
                pk2 = main.tile([128, BG, 128], FP, tag="pk2")
                Gp.tensor_tensor(pk2[:], dq2[:], iotaB, Alu.subtract)
                mx2 = main.tile([128, BG], FP, tag="mx2")
                V.tensor_reduce(mx2[:], pk2[:], Ax.X, Alu.max)
                mxb2 = main.tile([128, BG], FP, tag="mxb2")
                V.tensor_scalar(mxb2[:], mx2[:], -1.0, 1.0, Alu.mult, Alu.add)
                oh2 = main.tile([128, BG, 128], BF, tag="oh2")
                for i in range(BG):
                    S.activation(oh2[:, i, :], pk2[:, i, :], Act.Relu, bias=mxb2[:, i:i + 1])

                # -------- item1: t = clamp(round(g'), 0, 9) --------
                s2t = main.tile([128, BG, 128], FP, tag="s2t")
                S.activation(s2t[:], gps[:], Act.Copy, bias=M23)
                s3t = main.tile([128, BG, 128], FP, tag="s3t")
                S.activation(s3t[:], s2t[:], Act.Copy, bias=-M23)
                t = main.tile([128, BG, 128], FP, tag="t")
                V.tensor_scalar(t[:], s3t[:], 0.0, 9.0, Alu.max, Alu.min)
                hq = main.tile([128, BG, 128], FP, tag="hq")
                V.scalar_tensor_tensor(hq[:], gps[:], -2.0, t[:], Alu.mult, Alu.add)
                q = main.tile([128, BG, 128], FP, tag="q")
                Gp.tensor_tensor(q[:], hq[:], t[:], Alu.mult)
                vE = main.tile([128, BG, 128], FP, tag="vE")
                V.tensor_tensor(vE[:], q[:], erep[:], Alu.mult)
                dqA = main.tile([128, BG, 128], FP, tag="dqA")
                V.tensor_tensor(dqA[:], vE[:], ops[:], Alu.add)
                dq = main.tile([128, BG, 128], FP, tag="dq")
                S.activation(dq[:], dqA[:], Act.Copy, bias=-CQ)
                pkN = main.tile([128, BG, 128], FP, tag="pkN")
                Gp.tensor_tensor(pkN[:], dq[:], iotaB, Alu.subtract)
                mx = main.tile([128, BG], FP, tag="mx")
                V.tensor_reduce(mx[:], pkN[:], Ax.X, Alu.max)
                mxb1 = main.tile([128, BG], FP, tag="mxb1")
                V.tensor_scalar(mxb1[:], mx[:], -1.0, 1.0, Alu.mult, Alu.add)

                oh = main.tile([128, BG, 128], BF, tag="oh")
                for i in range(BG):
                    if i % 2 == 0:
                        V.tensor_scalar(oh[:, i, :], pkN[:, i, :], mx[:, i:i + 1], None, Alu.is_equal)
                    else:
                        S.activation(oh[:, i, :], pkN[:, i, :], Act.Relu, bias=mxb1[:, i:i + 1])

                # -------- XBAR-transpose one-hots, gather via matmul --------
                ohT = main.tile([128, BG, 128], BF, tag="ohT")
                oh2T = main.tile([128, BG, 128], BF, tag="oh2T")
                nc.sync.dma_start_transpose(ohT[:], oh[:].rearrange("m i n -> m (i n)"))
                nc.sync.dma_start_transpose(oh2T[:], oh2[:].rearrange("m i n -> m (i n)"))

                exPS = ps_ex.tile([128, BG, 14], FP, tag="exPS")
                for i in range(BG):
                    g = g0 + i
                    nc.tensor.matmul(exPS[:, i, 0:10], ohT[:, i, :], T12[:, g, :], start=True, stop=True)
                    nc.tensor.matmul(exPS[:, i, 10:14], oh2T[:, i, :], T3c[:, g, :], start=True, stop=True)
                ext = exA if b < 4 else exB
                S.activation(ext[:, (g0 % 16):(g0 % 16) + BG, :], exPS[:], Act.Copy)
                if b == 3:
                    _TAILA = tail_chunk(exA, 0, "A")

            # ---------------- tail (chunked, overlaps main loop) -------------
            s1a, s2a = _TAILA
            s1b, s2b = tail_chunk(exB, 16, "B")
            sboth = tl.tile([128, 2], FP, tag="sboth")
            V.tensor_tensor(sboth[:, 0:1], s1a[:], s1b[:], Alu.add)
            V.tensor_tensor(sboth[:, 1:2], s2a[:], s2b[:], Alu.add)
            sc_ps = ps_out.tile([2, 1], FP, tag="sc_ps")
            nc.tensor.matmul(sc_ps[:], sboth[:], onesc[:], start=True, stop=True)
            outsb = tl.tile([2, 1], FP, tag="outsb")
            V.tensor_copy(outsb[:], sc_ps[:])
            nc.sync.dma_start(out_d[:].rearrange("(a b) -> a b", b=1), outsb[:])
            ps_out_cm.__exit__(None, None, None)
            ps_ex_cm.__exit__(None, None, None)
            ps_d2_cm.__exit__(None, None, None)
            ps_grid_cm.__exit__(None, None, None)

    return nc


_CACHE = {}


def _get_program():
    if "nc" not in _CACHE:
        nc = bacc.Bacc("TRN2", target_bir_lowering=False, num_devices=N_CORES)
        pc_d = nc.declare_dram_parameter("pc", [G, P, 2], dt.float32, isOutput=False)
        po_d = nc.declare_dram_parameter("po", [G, P, 2], dt.float32, isOutput=False)
        gc_d = nc.declare_dram_parameter("gc", [G, P, 2], dt.float32, isOutput=False)
        gk_d = nc.declare_dram_parameter("gk", [G, P, 2], dt.float32, isOutput=False)
        mk_d = nc.declare_dram_parameter("mk", [G, P], dt.float32, isOutput=False)
        out_d = nc.declare_dram_parameter("out", [2], dt.float32, isOutput=True)
        _build(nc, pc_d[:], po_d[:], gc_d[:], gk_d[:], mk_d[:], out_d[:])
        nc.compile()
        _CACHE["nc"] = nc
    return _CACHE["nc"]


def _in_maps(inputs):
    pc = np.ascontiguousarray(inputs["pred_contours"], dtype=np.float32)
    po = np.ascontiguousarray(inputs["pred_offsets"], dtype=np.float32)
    gc = np.ascontiguousarray(inputs["gt_contours"], dtype=np.float32)
    gk = np.ascontiguousarray(inputs["gt_key_points"], dtype=np.float32)
    mk = np.ascontiguousarray(inputs["gt_key_points_mask"]).astype(np.float32)
    maps = []
    for c in range(N_CORES):
        s = slice(c * G, (c + 1) * G)
        maps.append({
            "pc": pc[s], "po": po[s], "gc": gc[s], "gk": gk[s], "mk": mk[s],
        })
    return maps


def kernel(pred_contours, pred_offsets, gt_contours, gt_key_points, gt_key_points_mask,
           _results_hook=None):
    inputs = {
        "pred_contours": pred_contours,
        "pred_offsets": pred_offsets,
        "gt_contours": gt_contours,
        "gt_key_points": gt_key_points,
        "gt_key_points_mask": gt_key_points_mask,
    }
    nc = _get_program()
    res = run_bass_kernel_spmd(nc, _in_maps(inputs), list(range(N_CORES)))
    if _results_hook is not None:
        _results_hook(res)
    s1 = f32(0.0)
    s2 = f32(0.0)
    for r in res.results:
        s1 = f32(s1 + f32(r["out"][0]))
        s2 = f32(s2 + f32(r["out"][1]))
    cnt1 = f32(N * P * 2)
    cnt2 = f32(max(float(np.sum(gt_key_points_mask != 0)) * 2.0, 1.0))
    loss = f32(f32(s1 / cnt1) * f32(0.5) + f32(s2 / cnt2) * f32(0.5))
    return np.asarray(loss, dtype=np.float32)


# revision 32
# speedup vs baseline: 1.0592x; 1.0592x over previous
"""Trainium2 Bass kernel for nn_DMLoss (contour matching loss), 8-core data parallel.

v4: block-diagonal bf16 split-precision matmuls with the C1 rounding offset
folded in as extra contraction rows, XBAR DMA-transposed one-hots, combined
packed argmin reduce, engine-balanced chain, piecewise prep DMA overlap.

Per instance (P=128 points, TIME=10):
  item1: nearest of 1280 interpolated gt points per pred point.  Segment n
    spans A_n = gt[n-1]..gt[n]; g' = 10*u - 0.5 with u = <p-A,D>/|D|^2; best
    discrete t = clamp(round(g'), 0, 9); dist^2 = |p-A|^2 + (e/100)*t*(t-2g').
    TensorE produces grids g', opsC1 = SC*(2<p,A> - |A|^2) + C1 and
    erep = -SC*e/100 (bf16 hi/lo split rows, C1 = C1H + C1L exactly).
    dq = (q*erep + opsC1) - CQ lands exactly on the 128-grid; pk = dq - n
    packs quantized distance + segment index; reduce-max = argmin;
    exact 0/1 one-hots gather segment data via XBAR transpose + bf16 matmul.
  item2: same machinery without interpolation (nearest pred per key point).

Output per core: [sum_loss1, sum_loss2]; host divides by counts and combines.
"""
import sys

for _p in ("/opt/trn_rl_repo",):
    if _p not in sys.path:
        sys.path.insert(0, _p)

import numpy as np

import concourse.bass as bass
import concourse.tile as tile
from concourse import bacc, mybir
from concourse.bass_utils import run_bass_kernel_spmd

dt = mybir.dt
Alu = mybir.AluOpType
Ax = mybir.AxisListType
Act = mybir.ActivationFunctionType
f32 = np.float32

N_CORES = 8
N, P = 256, 128
G = N // N_CORES          # instances per core = 32
BG = 4                    # instances per block
NB = G // BG              # 8 blocks
SC = 131072.0             # distance scale (quantum = 128/SC = 2^-10)
SHIFT = 48.0
BETA = 0.25               # smooth-l1 beta = 1/STRIDE
CQ = float(2 ** 30 + 2 ** 25)       # 1107296256
C1 = CQ - SC * SHIFT                # 1101004800
C1H = 1098907648.0                  # bf16-exact hi part of C1
C1L = 2097152.0                     # bf16-exact lo part (C1H + C1L == C1)
M23 = 8388608.0

# SPL slab regions, 10 row-types per operand (k = 10*i + t within a block):
#   lhsT: [x_hi, y_hi, x_lo, y_lo, x_hi2, y_hi2, 1, 1, 1, 1]
#   rhs:  [u_hi, v_hi, u_hi2, v_hi2, u_lo, v_lo, c_hi, c_lo, w8, w9]
# pairing t: xh*uh + yh*vh + xl*uh + yl*vh + xh*ul + yh*vl + ch + cl + w8 + w9
P0 = 0
K0 = 10
RG = 20         # w8 = w9 = 0
RO = 30         # w8, w9 = C1H, C1L
RQ = 40         # w8, w9 = C1H, C1L
RE = 50         # [z z z z z z er_hi er_lo z z]
TB = 60         # ax_hi ay_hi dx_hi dy_hi ox_hi oy_hi
NSLAB = 74
KB = 10 * BG    # block contraction rows = 40


def _build(nc, pc_d, po_d, gc_d, gk_d, mk_d, out_d):
    FP = dt.float32
    BF = dt.bfloat16

    with tile.TileContext(nc) as tc:
        with (
            tc.tile_pool(name="const", bufs=1) as cpool,
            tc.tile_pool(name="prep", bufs=1) as prep,
            tc.tile_pool(name="oper", bufs=1) as oper,
            tc.tile_pool(name="main", bufs=3) as main,
            tc.tile_pool(name="keep", bufs=1) as keep,
        ):
            V, Gp, S = nc.vector, nc.gpsimd, nc.scalar

            # ---------------- constants ----------------
            iota_i = cpool.tile([128, 128], dt.int32)
            Gp.iota(iota_i[:], pattern=[[1, 128]], channel_multiplier=0)
            iotaF = cpool.tile([128, 128], FP, tag="iotaF")
            V.tensor_copy(iotaF[:], iota_i[:])
            iotaC_i = cpool.tile([128, 1], dt.int32)
            Gp.iota(iotaC_i[:], pattern=[[0, 1]], channel_multiplier=1)
            iotaC = cpool.tile([128, 1], FP, tag="iotaC")
            V.tensor_copy(iotaC[:], iotaC_i[:])
            ident32F = cpool.tile([32, 32], FP, tag="ident32F")
            V.tensor_scalar(ident32F[:], iotaF[0:32, 0:32], iotaC[0:32], None, Alu.is_equal)
            onesc = cpool.tile([128, 1], FP, tag="onesc")
            Gp.memset(onesc[:], 1.0)

            # zeros region for rhs backfill
            zero_d = nc.dram_tensor("zeros", [KB, 32, 512], BF)
            ztile = prep.tile([KB, 512], BF, tag="ztile")
            V.memset(ztile[:], 0.0)
            nc.scalar.dma_start(zero_d[:, 0, :], ztile[:])
            nc.scalar.dma_start(zero_d[:, 1, :], zero_d[:, 0, :])
            nc.scalar.dma_start(zero_d[:, 2:4, :], zero_d[:, 0:2, :])
            nc.scalar.dma_start(zero_d[:, 4:8, :], zero_d[:, 0:4, :])
            nc.scalar.dma_start(zero_d[:, 8:16, :], zero_d[:, 0:8, :])
            nc.scalar.dma_start(zero_d[:, 16:32, :], zero_d[:, 0:16, :])

            # ---------------- contiguous input loads ----------------
            pc_i = prep.tile([32, 128, 2], FP, tag="pc_i")
            po_i = prep.tile([32, 128, 2], FP, tag="po_i")
            gc_i = prep.tile([32, 128, 2], FP, tag="gc_i")
            gk_i = prep.tile([32, 128, 2], FP, tag="gk_i")
            mk_i = prep.tile([32, 128], FP, tag="mk_i")
            a_i = prep.tile([32, 128, 2], FP, tag="a_i")
            nc.sync.dma_start(pc_i[:], pc_d[:, :, :])
            nc.sync.dma_start(po_i[:], po_d[:, :, :])
            nc.sync.dma_start(gc_i[:], gc_d[:, :, :])
            nc.scalar.dma_start(gk_i[:], gk_d[:, :, :])
            nc.scalar.dma_start(mk_i[:], mk_d[:, :])
            nc.sync.dma_start(a_i[:, 1:128, :], gc_d[:, 0:127, :])
            nc.scalar.dma_start(a_i[:, 0:1, :], gc_d[:, 127:128, :])

            # ---------------- SPL slabs + fine-grained stores ----------------
            SPL = prep.tile([32, NSLAB, 128], BF, tag="SPL")
            slab_d = nc.dram_tensor("slabs", [32, NSLAB, 128], BF)

            def split_pair(src_, s_hi, s_lo):
                # contiguous bf16 writes; channel-major strided reads
                srcT = src_.rearrange("g q c -> g c q")
                S.activation(SPL[:, s_hi:s_hi + 2, :], srcT, Act.Copy)
                if s_lo is not None:
                    V.tensor_tensor(SPL[:, s_lo:s_lo + 2, :], srcT,
                                    SPL[:, s_hi:s_hi + 2, :], Alu.subtract)

            def split_one(src_, s_hi, s_lo):
                S.activation(SPL[:, s_hi, :], src_, Act.Copy)
                if s_lo is not None:
                    V.tensor_tensor(SPL[:, s_lo, :], src_, SPL[:, s_hi, :], Alu.subtract)

            # ---- phase A: pc/gk-dependent slabs (no gc geometry needed) ----
            Gp.memset(SPL[:, P0 + 6:P0 + 10, :], 1.0)
            Gp.memset(SPL[:, K0 + 6:K0 + 10, :], 1.0)
            Gp.memset(SPL[:, RQ + 8, :], C1H)
            Gp.memset(SPL[:, RQ + 9, :], C1L)
            split_pair(pc_i[:], P0 + 0, P0 + 2)
            split_pair(gk_i[:], K0 + 0, K0 + 2)
            V.tensor_copy(SPL[:, P0 + 4:P0 + 6, :], SPL[:, P0 + 0:P0 + 2, :])
            V.tensor_copy(SPL[:, K0 + 4:K0 + 6, :], SPL[:, K0 + 0:K0 + 2, :])
            q_01 = prep.tile([32, 128, 2], FP, tag="q_01")
            S.activation(q_01[:], pc_i[:], Act.Copy, scale=2.0 * SC)
            psq = prep.tile([32, 128, 2], FP, tag="psq")
            Gp.tensor_tensor(psq[:], pc_i[:], pc_i[:], Alu.mult)
            zP = prep.tile([32, 128], FP, tag="zP")
            Gp.tensor_tensor(zP[:], psq[:, :, 0], psq[:, :, 1], Alu.add)
            q2 = prep.tile([32, 128], FP, tag="q2")
            S.activation(q2[:], zP[:], Act.Copy, scale=-SC)
            split_pair(q_01[:], RQ + 0, RQ + 4)
            split_one(q2[:], RQ + 6, RQ + 7)
            V.tensor_copy(SPL[:, RQ + 2:RQ + 4, :], SPL[:, RQ + 0:RQ + 2, :])
            nc.sync.dma_start(slab_d[:, 0:20, :], SPL[:, 0:20, :])
            nc.scalar.dma_start(slab_d[:, RQ:RQ + 10, :], SPL[:, RQ:RQ + 10, :])

            # ---- phase B: gc geometry ----
            d_i = prep.tile([32, 128, 2], FP, tag="d_i")
            V.tensor_tensor(d_i[:], gc_i[:], a_i[:], Alu.subtract)
            dsq = prep.tile([32, 128, 2], FP, tag="dsq")
            Gp.tensor_tensor(dsq[:], d_i[:], d_i[:], Alu.mult)
            e = prep.tile([32, 128], FP, tag="e")
            Gp.tensor_tensor(e[:], dsq[:, :, 0], dsq[:, :, 1], Alu.add)
            einv = prep.tile([32, 128], FP, tag="einv")
            V.reciprocal(einv[:], e[:])
            t_ad = prep.tile([32, 128, 2], FP, tag="t_ad")
            Gp.tensor_tensor(t_ad[:], a_i[:], d_i[:], Alu.mult)
            a2 = prep.tile([32, 128], FP, tag="a2")
            Gp.tensor_tensor(a2[:], t_ad[:, :, 0], t_ad[:, :, 1], Alu.add)
            asq = prep.tile([32, 128, 2], FP, tag="asq")
            Gp.tensor_tensor(asq[:], a_i[:], a_i[:], Alu.mult)
            zA = prep.tile([32, 128], FP, tag="zA")
            Gp.tensor_tensor(zA[:], asq[:, :, 0], asq[:, :, 1], Alu.add)

            er = prep.tile([32, 128], FP, tag="er")
            S.activation(er[:], e[:], Act.Copy, scale=-SC / 100.0)
            Gp.memset(SPL[:, RE + 0:RE + 6, :], 0.0)
            Gp.memset(SPL[:, RE + 8:RE + 10, :], 0.0)
            split_one(er[:], RE + 6, RE + 7)
            nc.sync.dma_start(slab_d[:, RE:RE + 10, :], SPL[:, RE:RE + 10, :])

            r_01 = prep.tile([32, 128, 2], FP, tag="r_01")
            V.scalar_tensor_tensor(r_01[:, :, 0], d_i[:, :, 0], 10.0, einv[:], Alu.mult, Alu.mult)
            V.scalar_tensor_tensor(r_01[:, :, 1], d_i[:, :, 1], 10.0, einv[:], Alu.mult, Alu.mult)
            r2 = prep.tile([32, 128], FP, tag="r2")
            V.scalar_tensor_tensor(r2[:], a2[:], -10.0, einv[:], Alu.mult, Alu.mult)
            Gp.memset(SPL[:, RG + 8:RG + 10, :], 0.0)
            split_pair(r_01[:], RG + 0, RG + 4)
            split_one(r2[:], RG + 6, RG + 7)
            V.tensor_copy(SPL[:, RG + 2:RG + 4, :], SPL[:, RG + 0:RG + 2, :])
            nc.scalar.dma_start(slab_d[:, RG:RG + 10, :], SPL[:, RG:RG + 10, :])

            o_01 = prep.tile([32, 128, 2], FP, tag="o_01")
            S.activation(o_01[:], a_i[:], Act.Copy, scale=2.0 * SC)
            o2 = prep.tile([32, 128], FP, tag="o2")
            S.activation(o2[:], zA[:], Act.Copy, scale=-SC)
            Gp.memset(SPL[:, RO + 8, :], C1H)
            Gp.memset(SPL[:, RO + 9, :], C1L)
            split_pair(o_01[:], RO + 0, RO + 4)
            split_one(o2[:], RO + 6, RO + 7)
            V.tensor_copy(SPL[:, RO + 2:RO + 4, :], SPL[:, RO + 0:RO + 2, :])
            nc.sync.dma_start(slab_d[:, RO:RO + 10, :], SPL[:, RO:RO + 10, :])

            # ---- table region ----
            split_pair(a_i[:], TB + 0, None)
            split_pair(d_i[:], TB + 2, None)
            split_pair(po_i[:], TB + 12, None)
            V.tensor_copy(SPL[:, TB + 4:TB + 6, :], SPL[:, RG + 0:RG + 5:4, :])
            V.tensor_copy(SPL[:, TB + 6:TB + 8, :], SPL[:, RG + 1:RG + 6:4, :])
            V.tensor_copy(SPL[:, TB + 8:TB + 10, :], SPL[:, RG + 6:RG + 8, :])
            V.tensor_copy(SPL[:, TB + 10:TB + 12, :], SPL[:, P0 + 0:P0 + 2, :])


            # lhsA[t, s, b, p] = slab (s=0 pred, s=1 key); rhsA[t, r, b, n]
            lhsA = oper.tile([KB, 2, 8, 128], BF, tag="lhsA")
            rhsA = oper.tile([KB, 4, 8, 512], BF, tag="rhsA")
            nc.sync.dma_start(
                rhsA[:], zero_d[:, :, :].rearrange("t (r b) n -> t r b n", r=4))

            for i in range(BG):
                for s in range(2):
                    eng = nc.sync if ((i + s) % 2 == 0) else nc.scalar
                    eng.dma_start(
                        lhsA[10 * i:10 * i + 10, s, :, :],
                        slab_d[i:32:4, 10 * s:10 * s + 10, :].rearrange("b t p -> t b p"),
                    )
                for r in range(4):
                    eng = nc.sync if ((i + r) % 2 == 0) else nc.scalar
                    eng.dma_start(
                        rhsA[10 * i:10 * i + 10, r, :, 128 * i:128 * (i + 1)],
                        slab_d[i:32:4, 20 + 10 * r:30 + 10 * r, :].rearrange("b t p -> t b p"),
                    )

            # ---------------- gather tables (one batched XBAR transpose) -----
            # in (32, 14*128) -> out stgB[n, j, g] = SPL[g, TB+j, n]
            T12 = keep.tile([128, G, 10], BF, tag="T12")
            T3c = keep.tile([128, G, 4], BF, tag="T3c")
            stgB = keep.tile([128, 14, 32], BF, tag="stgB")
            nc.scalar.dma_start_transpose(
                stgB[:], SPL[:, TB:TB + 14, :].rearrange("g j n -> g (j n)"))
            # T12 slots: [ax, ay, dx, dy, r0h, r0l, r1h, r1l, r2h, r2l] = j 0..9
            V.tensor_copy(T12[:], stgB[:, 0:10, :].rearrange("n j g -> n g j"))
            # T3c slots: [px, py, ox, oy] = j 10..13
            V.tensor_copy(T3c[:], stgB[:, 10:14, :].rearrange("n j g -> n g j"))

            # ---------------- f32 transposes for the tail --------------------
            pxP = keep.tile([128, G], FP, tag="pxP")
            pyP = keep.tile([128, G], FP, tag="pyP")
            oxP = keep.tile([128, G], FP, tag="oxP")
            oyP = keep.tile([128, G], FP, tag="oyP")
            kxP = keep.tile([128, G], FP, tag="kxP")
            kyP = keep.tile([128, G], FP, tag="kyP")
            mkP = keep.tile([128, G], FP, tag="mkP")
            with tc.tile_pool(name="ps_prep", bufs=3, space="PSUM") as ps_prep:
                for dst, src in ((pxP, pc_i[:, :, 0]), (pyP, pc_i[:, :, 1]),
                                 (oxP, po_i[:, :, 0]), (oyP, po_i[:, :, 1]),
                                 (kxP, gk_i[:, :, 0]), (kyP, gk_i[:, :, 1]),
                                 (mkP, mk_i[:])):
                    fps = ps_prep.tile([128, 32], FP, tag="tpsF")
                    nc.tensor.transpose(fps[:], src, ident32F[:])
                    S.activation(dst[:], fps[:], Act.Copy)

            exA = keep.tile([128, 16, 14], FP, tag="exA")
            exB = keep.tile([128, 16, 14], FP, tag="exB")
            tl = prep

            def tail_chunk(ex, gl, sfx):
                gs = slice(gl, gl + 16)

                def TT(name, a, bb, op, eng=V):
                    r = tl.tile([128, 16], FP, tag=sfx + name)
                    eng.tensor_tensor(r[:], a, bb, op)
                    return r

                r0 = TT("r0", ex[:, :, 4], ex[:, :, 5], Alu.add)
                r1 = TT("r1", ex[:, :, 6], ex[:, :, 7], Alu.add, Gp)
                r2t = TT("r2t", ex[:, :, 8], ex[:, :, 9], Alu.add)
                v1 = TT("v1", pxP[:, gs], r0[:], Alu.mult, Gp)
                v2 = TT("v2", pyP[:, gs], r1[:], Alu.mult)
                gst = TT("gst", v1[:], v2[:], Alu.add, Gp)
                gst = TT("gst2", gst[:], r2t[:], Alu.add)
                c2t = tl.tile([128, 16], FP, tag=sfx + "c2t")
                S.activation(c2t[:], gst[:], Act.Copy, bias=M23)
                c3t = tl.tile([128, 16], FP, tag=sfx + "c3t")
                S.activation(c3t[:], c2t[:], Act.Copy, bias=-M23)
                tst = tl.tile([128, 16], FP, tag=sfx + "tst")
                V.tensor_scalar(tst[:], c3t[:], 0.0, 9.0, Alu.max, Alu.min)
                m1 = TT("m1", tst[:], ex[:, :, 2], Alu.mult, Gp)
                tgx = tl.tile([128, 16], FP, tag=sfx + "tgx")
                V.scalar_tensor_tensor(tgx[:], m1[:], 0.1, ex[:, :, 0], Alu.mult, Alu.add)
                m2 = TT("m2", tst[:], ex[:, :, 3], Alu.mult, Gp)
                tgy = tl.tile([128, 16], FP, tag=sfx + "tgy")
                V.scalar_tensor_tensor(tgy[:], m2[:], 0.1, ex[:, :, 1], Alu.mult, Alu.add)

                def smooth_l1_sum(pred_x, pred_y, tx, ty, px_, py_, name):
                    acc = None
                    for ci, (pr, tt_, pp) in enumerate(((pred_x, tx, px_), (pred_y, ty, py_))):
                        s2fx = sfx + name + str(ci)
                        e1 = TT(name + str(ci) + "e1", tt_, pp, Alu.subtract, Gp)
                        dfe = tl.tile([128, 16], FP, tag=s2fx + "dfe")
                        V.scalar_tensor_tensor(dfe[:], e1[:], -0.25, pr, Alu.mult, Alu.add)
                        ad = tl.tile([128, 16], FP, tag=s2fx + "ad")
                        S.activation(ad[:], dfe[:], Act.Abs)
                        m = tl.tile([128, 16], FP, tag=s2fx + "m")
                        V.tensor_scalar(m[:], ad[:], BETA, None, Alu.min)
                        uu = tl.tile([128, 16], FP, tag=s2fx + "u")
                        V.scalar_tensor_tensor(uu[:], m[:], -0.5, ad[:], Alu.mult, Alu.add)
                        sl = tl.tile([128, 16], FP, tag=s2fx + "sl")
                        V.scalar_tensor_tensor(sl[:], m[:], 4.0, uu[:], Alu.mult, Alu.mult)
                        if acc is None:
                            acc = sl
                        else:
                            acc = TT(name + "acc", acc[:], sl[:], Alu.add, Gp)
                    return acc

                s1 = smooth_l1_sum(oxP[:, gs], oyP[:, gs], tgx[:], tgy[:],
                                   pxP[:, gs], pyP[:, gs], "i1")
                s2 = smooth_l1_sum(ex[:, :, 12], ex[:, :, 13], kxP[:, gs], kyP[:, gs],
                                   ex[:, :, 10], ex[:, :, 11], "i2")
                s2 = TT("s2m", s2[:], mkP[:, gs], Alu.mult)
                s1r = tl.tile([128, 1], FP, tag=sfx + "s1r")
                V.tensor_reduce(s1r[:], s1[:], Ax.X, Alu.add)
                s2r = tl.tile([128, 1], FP, tag=sfx + "s2r")
                V.tensor_reduce(s2r[:], s2[:], Ax.X, Alu.add)
                return s1r, s2r


            iotaB = iotaF[:].rearrange("p (o q) -> p o q", o=1).broadcast_to([128, BG, 128])

            ps_grid_cm = tc.tile_pool(name="ps_grid", bufs=1, space="PSUM")
            ps_d2_cm = tc.tile_pool(name="ps_d2", bufs=2, space="PSUM")
            ps_ex_cm = tc.tile_pool(name="ps_ex", bufs=2, space="PSUM")
            ps_out_cm = tc.tile_pool(name="ps_out", bufs=1, space="PSUM")
            ps_grid = ps_grid_cm.__enter__()
            ps_d2 = ps_d2_cm.__enter__()
            ps_ex = ps_ex_cm.__enter__()
            ps_out = ps_out_cm.__enter__()

            # ---------------- main loop ----------------
            for b in range(NB):
                g0 = b * BG
                gps = ps_grid.tile([128, BG, 128], FP, tag="gps")
                ops = ps_grid.tile([128, BG, 128], FP, tag="ops")
                erep = ps_grid.tile([128, BG, 128], FP, tag="erep")
                d2ps = ps_d2.tile([128, BG, 128], FP, tag="d2ps")
                gv = gps[:].rearrange("p i n -> p (i n)")
                ov = ops[:].rearrange("p i n -> p (i n)")
                ev = erep[:].rearrange("p i n -> p (i n)")
                dv = d2ps[:].rearrange("p i n -> p (i n)")
                nc.tensor.matmul(gv, lhsA[:, 0, b, :], rhsA[:, 0, b, :], start=True, stop=True)
                nc.tensor.matmul(ov, lhsA[:, 0, b, :], rhsA[:, 1, b, :], start=True, stop=True)
                nc.tensor.matmul(ev, lhsA[:, 0, b, :], rhsA[:, 3, b, :], start=True, stop=True)
                nc.tensor.matmul(dv, lhsA[:, 1, b, :], rhsA[:, 2, b, :], start=True, stop=True)

                # -------- item1: t = clamp(round(g'), 0, 9) --------
                s2t = main.tile([128, BG, 128], FP, tag="s2t")
                S.activation(s2t[:], gps[:], Act.Copy, bias=M23)
                s3t = main.tile([128, BG, 128], FP, tag="s3t")
                S.activation(s3t[:], s2t[:], Act.Copy, bias=-M23)
                t = main.tile([128, BG, 128], FP, tag="t")
                V.tensor_scalar(t[:], s3t[:], 0.0, 9.0, Alu.max, Alu.min)
                hq = main.tile([128, BG, 128], FP, tag="hq")
                V.scalar_tensor_tensor(hq[:], gps[:], -2.0, t[:], Alu.mult, Alu.add)
                q = main.tile([128, BG, 128], FP, tag="q")
                Gp.tensor_tensor(q[:], hq[:], t[:], Alu.mult)
                vE = main.tile([128, BG, 128], FP, tag="vE")
                V.tensor_tensor(vE[:], q[:], erep[:], Alu.mult)
                dqA = main.tile([128, BG, 128], FP, tag="dqA")
                V.tensor_tensor(dqA[:], vE[:], ops[:], Alu.add)
                dq = main.tile([128, BG, 128], FP, tag="dq")
                S.activation(dq[:], dqA[:], Act.Copy, bias=-CQ)
                dq2 = main.tile([128, BG, 128], FP, tag="dq2")
                S.activation(dq2[:], d2ps[:], Act.Copy, bias=-CQ)

                pkN = main.tile([128, BG, 128], FP, tag="pkN")
                Gp.tensor_tensor(pkN[:], dq[:], iotaB, Alu.subtract)
                pk2 = main.tile([128, BG, 128], FP, tag="pk2")
                Gp.tensor_tensor(pk2[:], dq2[:], iotaB, Alu.subtract)
                mx = main.tile([128, BG], FP, tag="mx")
                V.tensor_reduce(mx[:], pkN[:], Ax.X, Alu.max)
                mx2 = main.tile([128, BG], FP, tag="mx2")
                V.tensor_reduce(mx2[:], pk2[:], Ax.X, Alu.max)
                mxb1 = main.tile([128, BG], FP, tag="mxb1")
                V.tensor_scalar(mxb1[:], mx[:], -1.0, 1.0, Alu.mult, Alu.add)
                mxb2 = main.tile([128, BG], FP, tag="mxb2")
                V.tensor_scalar(mxb2[:], mx2[:], -1.0, 1.0, Alu.mult, Alu.add)

                oh = main.tile([128, BG, 128], BF, tag="oh")
                oh2 = main.tile([128, BG, 128], BF, tag="oh2")
                for i in range(BG):
                    if i % 2 == 0:
                        V.tensor_scalar(oh[:, i, :], pkN[:, i, :], mx[:, i:i + 1], None, Alu.is_equal)
                    else:
                        S.activation(oh[:, i, :], pkN[:, i, :], Act.Relu, bias=mxb1[:, i:i + 1])
                    S.activation(oh2[:, i, :], pk2[:, i, :], Act.Relu, bias=mxb2[:, i:i + 1])

                # -------- XBAR-transpose one-hots, gather via matmul --------
                ohT = main.tile([128, BG, 128], BF, tag="ohT")
                oh2T = main.tile([128, BG, 128], BF, tag="oh2T")
                nc.sync.dma_start_transpose(ohT[:], oh[:].rearrange("m i n -> m (i n)"))
                nc.sync.dma_start_transpose(oh2T[:], oh2[:].rearrange("m i n -> m (i n)"))

                exPS = ps_ex.tile([128, BG, 14], FP, tag="exPS")
                for i in range(BG):
                    g = g0 + i
                    nc.tensor.matmul(exPS[:, i, 0:10], ohT[:, i, :], T12[:, g, :], start=True, stop=True)
                    nc.tensor.matmul(exPS[:, i, 10:14], oh2T[:, i, :], T3c[:, g, :], start=True, stop=True)
                ext = exA if b < 4 else exB
                S.activation(ext[:, (g0 % 16):(g0 % 16) + BG, :], exPS[:], Act.Copy)
                if b == 3:
                    _TAILA = tail_chunk(exA, 0, "A")

            # ---------------- tail (chunked, overlaps main loop) -------------
            s1a, s2a = _TAILA
            s1b, s2b = tail_chunk(exB, 16, "B")
            sboth = tl.tile([128, 2], FP, tag="sboth")
            V.tensor_tensor(sboth[:, 0:1], s1a[:], s1b[:], Alu.add)
            V.tensor_tensor(sboth[:, 1:2], s2a[:], s2b[:], Alu.add)
            sc_ps = ps_out.tile([2, 1], FP, tag="sc_ps")
            nc.tensor.matmul(sc_ps[:], sboth[:], onesc[:], start=True, stop=True)
            outsb = tl.tile([2, 1], FP, tag="outsb")
            V.tensor_copy(outsb[:], sc_ps[:])
            nc.sync.dma_start(out_d[:].rearrange("(a b) -> a b", b=1), outsb[:])
            ps_out_cm.__exit__(None, None, None)
            ps_ex_cm.__exit__(None, None, None)
            ps_d2_cm.__exit__(None, None, None)
            ps_grid_cm.__exit__(None, None, None)

    return nc


_CACHE = {}


def _get_program():
    if "nc" not in _CACHE:
        nc = bacc.Bacc("TRN2", target_bir_lowering=False, num_devices=N_CORES)
        pc_d = nc.declare_dram_parameter("pc", [G, P, 2], dt.float32, isOutput=False)
        po_d = nc.declare_dram_parameter("po", [G, P, 2], dt.float32, isOutput=False)
        gc_d = nc.declare_dram_parameter("gc", [G, P, 2], dt.float32, isOutput=False)
        gk_d = nc.declare_dram_parameter("gk", [G, P, 2], dt.float32, isOutput=False)
        mk_d = nc.declare_dram_parameter("mk", [G, P], dt.float32, isOutput=False)
        out_d = nc.declare_dram_parameter("out", [2], dt.float32, isOutput=True)
        _build(nc, pc_d[:], po_d[:], gc_d[:], gk_d[:], mk_d[:], out_d[:])
        nc.compile()
        _CACHE["nc"] = nc
    return _CACHE["nc"]


def _in_maps(inputs):
    pc = np.ascontiguousarray(inputs["pred_contours"], dtype=np.float32)
    po = np.ascontiguousarray(inputs["pred_offsets"], dtype=np.float32)
    gc = np.ascontiguousarray(inputs["gt_contours"], dtype=np.float32)
    gk = np.ascontiguousarray(inputs["gt_key_points"], dtype=np.float32)
    mk = np.ascontiguousarray(inputs["gt_key_points_mask"]).astype(np.float32)
    maps = []
    for c in range(N_CORES):
        s = slice(c * G, (c + 1) * G)
        maps.append({
            "pc": pc[s], "po": po[s], "gc": gc[s], "gk": gk[s], "mk": mk[s],
        })
    return maps


def kernel(pred_contours, pred_offsets, gt_contours, gt_key_points, gt_key_points_mask,
           _results_hook=None):
    inputs = {
        "pred_contours": pred_contours,
        "pred_offsets": pred_offsets,
        "gt_contours": gt_contours,
        "gt_key_points": gt_key_points,
        "gt_key_points_mask": gt_key_points_mask,
    }
    nc = _get_program()
    res = run_bass_kernel_spmd(nc, _in_maps(inputs), list(range(N_CORES)))
    if _results_hook is not None:
        _results_hook(res)
    s1 = f32(0.0)
    s2 = f32(0.0)
    for r in res.results:
        s1 = f32(s1 + f32(r["out"][0]))
        s2 = f32(s2 + f32(r["out"][1]))
    cnt1 = f32(N * P * 2)
    cnt2 = f32(max(float(np.sum(gt_key_points_mask != 0)) * 2.0, 1.0))
    loss = f32(f32(s1 / cnt1) * f32(0.5) + f32(s2 / cnt2) * f32(0.5))
    return np.asarray(loss, dtype=np.float32)
